# revision 20
# baseline (speedup 1.0000x reference)
"""Contrastive loss kernel for Trainium2, 8 NeuronCores, data-parallel over node rows.

v2 strategy (per core c, shard rows R_c = c*1024 .. c*1024+1024), gather-free:
  - Gram matrix in fp8(e4m3) with DoubleRow matmuls (K=256 folded as 128x2).
    The DR pair axis maps d = 2*q + j, which falls out of a uint16-viewed DMA
    transpose of fp8 element pairs -- no separate cast pass.
  - Mask application ON THE PE: an identity-weights fp8 matmul accumulates the
    ln-mask (0 selected / -20 unselected) straight into the Gram PSUM chunk, so
    selection is a single ACT pass: exp(2*(sim+mask)) with accum_out giving the
    per-row masked sums directly from PSUM.  A few chunks per column use the
    DVE add-first path instead to balance engine load.
  - Full-x normalize streamed in groups; rsqrt via DVE polynomial + one Newton
    step (SS/256 in [0.66,1.37]) so the Scalar activation table never thrashes
    between Sqrt and Exp mid-stream.
  - Chunk-column-major slab loop so Gram consumption tracks the group stream.
  - Positives: fused DVE dots on bf16 shard tiles; ln(pos) taken analytically.
  - loss = ln(pos + neg + eps) - 2*pos_arg; host averages the 8 cores' rows.
"""
import sys

sys.path.insert(0, "/opt/trn_rl_repo")

from contextlib import ExitStack

import numpy as np
import ml_dtypes

import concourse.bacc as bacc
import concourse.mybir as mybir
import concourse.tile as tile
from concourse.bass_utils import run_bass_kernel_spmd

N_NODES = 8192
D = 256
K_NEG = 64
N_CORES = 8
RPC = N_NODES // N_CORES      # rows per core = 1024
SLABS = RPC // 128            # 8 slabs of 128 rows
NT = N_NODES // 128           # 64 x-tiles of [128, 256]
NTS = RPC // 128              # 8 shard tiles
G = 8                         # tiles per x-group
NG = NT // G                  # 8 groups
CHUNK = 2048                  # Gram chunk (4 PSUM banks)
NCH = N_NODES // CHUNK        # 4 chunks per slab row
TAU_INV = 2.0                 # 1/(0.5 + 1e-10) ~= 2.0
EPS = 1e-5

F32 = mybir.dt.float32
BF16 = mybir.dt.bfloat16
FP8 = mybir.dt.float8e4
U16 = mybir.dt.uint16
MASK_NP = ml_dtypes.float8_e4m3

# per-chunk selection mode, keyed (slab, chunk).
# mask m01inv is 1.0 for UNSELECTED cols, 0.0 for negatives (and fp8-exact).
#   S: PE adds -20*m01inv via (-20*I)-matmul; ACT exp(2*in) from PSUM w/ accum.
#   V: DVE scalar_tensor_tensor Y = -20*m01inv + sim (bf16); ACT exp w/ accum.
# Early columns lean S (DVE busy with preamble); later columns lean V.
_MODE_COLS = [
    "SSSSSSSS",   # ch 0, slabs 0..7
    "SSVVSSVV",   # ch 1
    "VSVVSVVS",   # ch 2
    "VVSVVSVV",   # ch 3
]
MODE = {(s, ch): _MODE_COLS[ch][s] for ch in range(4) for s in range(8)}

_PROG = None


def _poly_rsqrt(nc, big, SSg, Rout):
    """Rout = 1/sqrt(SSg) for SSg ~ 256, DVE-only (no ACT table).

    a = SS/256; t = a-1; y0 = 1 - t/2 + 0.375 t^2; one Newton step.
    Scaled: 1/sqrt(SS) = (1/16) / sqrt(a) -> fold 1/16 into the last mul.
    """
    ALU = mybir.AluOpType
    shp = list(SSg.shape)
    t = big.tile(shp, F32, tag="rs_t")
    nc.vector.tensor_scalar(out=t, in0=SSg, scalar1=1.0 / 256.0, scalar2=-1.0,
                            op0=ALU.mult, op1=ALU.add)
    u = big.tile(shp, F32, tag="rs_u")
    nc.vector.tensor_scalar(out=u, in0=t, scalar1=0.375, scalar2=-0.5,
                            op0=ALU.mult, op1=ALU.add)
    w = big.tile(shp, F32, tag="rs_w")
    nc.vector.tensor_mul(w, u, t)
    y0 = big.tile(shp, F32, tag="rs_y0")
    nc.vector.tensor_scalar(out=y0, in0=w, scalar1=1.0, scalar2=None,
                            op0=ALU.add)
    b = big.tile(shp, F32, tag="rs_b")
    nc.vector.tensor_scalar(out=b, in0=SSg, scalar1=1.0 / 512.0, scalar2=None,
                            op0=ALU.mult)
    c = big.tile(shp, F32, tag="rs_c")
    nc.vector.tensor_mul(c, y0, y0)
    d = big.tile(shp, F32, tag="rs_d")
    nc.vector.tensor_mul(d, b, c)
    e = big.tile(shp, F32, tag="rs_e")
    nc.vector.tensor_scalar(out=e, in0=d, scalar1=-1.0, scalar2=1.5,
                            op0=ALU.mult, op1=ALU.add)
    y1 = big.tile(shp, F32, tag="rs_y1")
    nc.vector.tensor_mul(y1, y0, e)
    # R = y1 / 16
    nc.vector.tensor_scalar(out=Rout, in0=y1, scalar1=1.0 / 16.0, scalar2=None,
                            op0=ALU.mult)


def _build_program():
    nc = bacc.Bacc("TRN2", target_bir_lowering=False, debug=False,
                   num_devices=N_CORES)

    xb_d = nc.dram_tensor("xb", [N_NODES, D], BF16, kind="ExternalInput")
    xsh_d = nc.dram_tensor("xsh", [RPC, D], BF16, kind="ExternalInput")
    ys_d = nc.dram_tensor("ys", [RPC, D], BF16, kind="ExternalInput")
    lnm_d = nc.dram_tensor("lnm", [128, SLABS, N_NODES], FP8,
                           kind="ExternalInput")
    id_d = nc.dram_tensor("idf8", [128, 128], FP8, kind="ExternalInput")
    loss_d = nc.dram_tensor("loss", [128, SLABS], F32, kind="ExternalOutput")

    AF = mybir.ActivationFunctionType
    ALU = mybir.AluOpType
    DR = mybir.MatmulPerfMode.DoubleRow

    with tile.TileContext(nc) as tc, ExitStack() as ctx:
        big = ctx.enter_context(tc.tile_pool(name="big", bufs=1))
        sqpool = ctx.enter_context(tc.tile_pool(name="sqpool", bufs=1))
        zpool = ctx.enter_context(tc.tile_pool(name="zpool", bufs=2))
        ypool = ctx.enter_context(tc.tile_pool(name="ypool", bufs=2))
        expool = ctx.enter_context(tc.tile_pool(name="expool", bufs=2))
        psum = ctx.enter_context(tc.tile_pool(name="psum", bufs=2, space="PSUM"))

        # ---------------- input DMAs (x before masks!) ----------------
        xshb = big.tile([128, NTS, D], BF16)
        nc.sync.dma_start(out=xshb,
                          in_=xsh_d.ap().rearrange("(t p) d -> p t d", p=128))
        ysb = big.tile([128, NTS, D], BF16)
        nc.sync.dma_start(out=ysb,
                          in_=ys_d.ap().rearrange("(t p) d -> p t d", p=128))
        idsb = big.tile([128, 128], FP8)
        nc.sync.dma_start(out=idsb, in_=id_d.ap())
        x_r = xb_d.ap().rearrange("(t p) d -> p t d", p=128)
        xall = big.tile([128, NT, D], BF16)
        lnm = big.tile([128, SLABS, N_NODES], FP8)
        # queue split: big loads stay off the sync queue so the DMA
        # transposes are not head-of-line blocked behind them.
        for s in range(4):
            nc.scalar.dma_start(out=lnm[:, s, :], in_=lnm_d.ap()[:, s, :])
        for g in range(NG):
            nc.gpsimd.dma_start(out=xall[:, g * G:(g + 1) * G, :],
                                in_=x_r[:, g * G:(g + 1) * G, :])
            if g < 4:
                s = 4 + g
                nc.gpsimd.dma_start(out=lnm[:, s, :], in_=lnm_d.ap()[:, s, :])

        # ---------------- shard norms + positives ----------------
        SSx = big.tile([128, NTS], BF16)
        sqs = sqpool.tile([128, NTS, D], BF16, tag="sqs")
        nc.vector.tensor_mul(sqs.rearrange("p a b -> p (a b)"),
                             xshb.rearrange("p a b -> p (a b)"),
                             xshb.rearrange("p a b -> p (a b)"))
        with nc.allow_low_precision(reason="bf16 norm sums, 4x DVE mode"):
            nc.vector.tensor_reduce(out=SSx, in_=sqs,
                                    axis=mybir.AxisListType.X, op=ALU.add)
        SSy = big.tile([128, NTS], BF16)
        sqy = sqpool.tile([128, NTS, D], BF16, tag="sqs")
        nc.vector.tensor_mul(sqy.rearrange("p a b -> p (a b)"),
                             ysb.rearrange("p a b -> p (a b)"),
                             ysb.rearrange("p a b -> p (a b)"))
        with nc.allow_low_precision(reason="bf16 norm sums, 4x DVE mode"):
            nc.vector.tensor_reduce(out=SSy, in_=sqy,
                                    axis=mybir.AxisListType.X, op=ALU.add)
        SSxi = big.tile([128, NTS], F32)
        nc.vector.reciprocal(SSxi, SSx)
        SSyi = big.tile([128, NTS], F32)
        nc.vector.reciprocal(SSyi, SSy)
        Rx = big.tile([128, NTS], F32)
        nc.scalar.activation(Rx, SSxi, AF.Sqrt)
        Ry = big.tile([128, NTS], F32)
        nc.scalar.activation(Ry, SSyi, AF.Sqrt)

        # positive dots
        xy = sqpool.tile([128, NTS, D], BF16, tag="sqs")
        nc.vector.tensor_mul(xy.rearrange("p a b -> p (a b)"),
                             xshb.rearrange("p a b -> p (a b)"),
                             ysb.rearrange("p a b -> p (a b)"))
        DXY = big.tile([128, NTS], BF16)
        with nc.allow_low_precision(reason="bf16 pos dots, 4x DVE mode"):
            nc.vector.tensor_reduce(out=DXY, in_=xy,
                                    axis=mybir.AxisListType.X, op=ALU.add)
        PA = big.tile([128, NTS], F32)
        nc.vector.tensor_mul(PA, DXY, Rx)
        PA2 = big.tile([128, NTS], F32)
        nc.vector.tensor_mul(PA2, PA, Ry)
        POS = big.tile([128, NTS], F32)
        nc.scalar.activation(POS, PA2, AF.Exp, scale=TAU_INV)

        # ---------------- shard z (fp8 pairs) + transpose ----------------
        zsf8 = big.tile([128, NTS, D], FP8)
        nc.gpsimd.tensor_tensor(
            out=zsf8, in0=xshb,
            in1=Rx.unsqueeze(2).broadcast_to((128, NTS, D)), op=ALU.mult)
        z1sT = big.tile([128, 2 * RPC], FP8)   # u16 cols = RPC
        nc.sync.dma_start(
            out=z1sT.bitcast(U16).rearrange("p (b q) -> p b q", q=128),
            in_=zsf8.bitcast(U16).rearrange("p a b -> p (a b)"),
            transpose=True)
        # de-interleave pairs for the weights: LDWEIGHTS DoubleRow requires the
        # pair stride to be a multiple of 16 elements (s3_lw_dual_fp8), so the
        # adjacent-pair transpose layout is illegal for lhsT. One DVE copy.
        z1sT_w = big.tile([128, 2, RPC], FP8)
        nc.vector.tensor_copy(out=z1sT_w,
                              in_=z1sT.rearrange("p (n j) -> p j n", j=2))

        # ---------------- full-x: norms (DVE rsqrt) + scale + transpose ----
        z1T = big.tile([128, 2 * N_NODES], FP8)  # u16 cols = N_NODES
        R = big.tile([128, NT], F32)
        SS = big.tile([128, NT], BF16)
        for g in range(NG):
            sl = slice(g * G, (g + 1) * G)
            sq = sqpool.tile([128, G, D], BF16, tag="sqg")
            nc.vector.tensor_mul(sq.rearrange("p a b -> p (a b)"),
                                 xall[:, sl, :].rearrange("p a b -> p (a b)"),
                                 xall[:, sl, :].rearrange("p a b -> p (a b)"))
            with nc.allow_low_precision(reason="bf16 norm sums, 4x DVE mode"):
                nc.vector.tensor_reduce(out=SS[:, sl], in_=sq,
                                        axis=mybir.AxisListType.X, op=ALU.add)
            _poly_rsqrt(nc, zpool, SS[:, sl], R[:, sl])
            zf8 = zpool.tile([128, G, D], FP8, tag="zf8")
            nc.gpsimd.tensor_tensor(
                out=zf8, in0=xall[:, sl, :],
                in1=R[:, sl].unsqueeze(2).broadcast_to((128, G, D)),
                op=ALU.mult)
            nc.sync.dma_start(
                out=z1T.bitcast(U16)[:, g * G * 128:(g + 1) * G * 128]
                       .rearrange("p (b q) -> p b q", q=128),
                in_=zf8.bitcast(U16).rearrange("p a b -> p (a b)"),
                transpose=True)
        z1T_dr = z1T.rearrange("p (n j) -> p j n", j=2)  # [128, 2, N_NODES]

        # ---------------- selection: chunk-column-major slab loop ----------
        NEGC = big.tile([128, SLABS * NCH], F32)
        nc.vector.memset(NEGC, 0.0)
        for ch in range(NCH):
            for s in range(SLABS):
                mode = MODE[(s, ch)]
                lhsT = z1sT_w[:, :, s * 128:(s + 1) * 128]
                acc = NEGC[:, s * NCH + ch:s * NCH + ch + 1]
                msk = lnm[:, s, ch * CHUNK:(ch + 1) * CHUNK]
                ps = psum.tile([128, CHUNK], F32, tag="ps")
                for j in range(CHUNK // 512):
                    col = ch * CHUNK + j * 512
                    nc.tensor.matmul(ps[:, j * 512:(j + 1) * 512],
                                     lhsT=lhsT,
                                     rhs=z1T_dr[:, :, col:col + 512],
                                     start=True, stop=(mode != "S"),
                                     perf_mode=DR)
                if mode == "S":
                    # PSUM += 20*m01 via identity matmul; exp(2*in - 40)
                    for j in range(CHUNK // 512):
                        col = ch * CHUNK + j * 512
                        nc.tensor.matmul(ps[:, j * 512:(j + 1) * 512],
                                         lhsT=idsb,
                                         rhs=lnm[:, s, col:col + 512],
                                         start=False, stop=True)
                    ex = expool.tile([128, CHUNK], BF16, tag="ex")
                    nc.scalar.activation(ex, ps, AF.Exp, scale=TAU_INV,
                                         accum_out=acc)
                elif mode == "V":
                    Y = ypool.tile([128, CHUNK], BF16, tag="y")
                    nc.vector.scalar_tensor_tensor(
                        out=Y, in0=msk, scalar=-20.0, in1=ps,
                        op0=ALU.mult, op1=ALU.add)
                    ex = expool.tile([128, CHUNK], BF16, tag="ex")
                    nc.scalar.activation(ex, Y, AF.Exp, scale=TAU_INV,
                                         accum_out=acc)

        # ---------------- loss assembly ----------------
        NEG = big.tile([128, SLABS], F32)
        nc.vector.tensor_reduce(
            out=NEG, in_=NEGC.rearrange("p (s c) -> p s c", c=NCH),
            axis=mybir.AxisListType.X, op=ALU.add)
        DEN = big.tile([128, SLABS], F32)
        nc.vector.tensor_add(DEN, NEG, POS)
        DEN2 = big.tile([128, SLABS], F32)
        nc.vector.tensor_scalar_add(DEN2, DEN, EPS)
        LD = big.tile([128, SLABS], F32)
        nc.scalar.activation(LD, DEN2, AF.Ln)
        LP = big.tile([128, SLABS], F32)
        nc.vector.tensor_scalar_mul(LP, PA2, TAU_INV)
        LOSS = big.tile([128, SLABS], F32)
        nc.vector.tensor_sub(LOSS, LD, LP)
        nc.sync.dma_start(out=loss_d.ap(), in_=LOSS)

    nc.compile()
    return nc


def _get_program():
    global _PROG
    if _PROG is None:
        _PROG = _build_program()
    return _PROG


def _make_mask(idx_core: np.ndarray) -> np.ndarray:
    """[1024, 64] int -> [128, 8, 8192] fp8 0/1 mask (p, slab, col)."""
    idxc = idx_core.reshape(SLABS, 128, K_NEG).transpose(1, 0, 2)  # [p, s, k]
    m01 = np.ones((128, SLABS, N_NODES), dtype=np.float32)
    pp = np.arange(128)[:, None, None]
    ss = np.arange(SLABS)[None, :, None]
    m01[pp, ss, idxc] = 0.0
    return m01.astype(MASK_NP)


def make_in_maps(x, y, neg_indices):
    xb = np.ascontiguousarray(x).astype(ml_dtypes.bfloat16)
    idf8 = (-20.0 * np.eye(128, dtype=np.float32)).astype(MASK_NP)
    in_maps = []
    for c in range(N_CORES):
        lo, hi = c * RPC, (c + 1) * RPC
        in_maps.append({
            "xb": xb,
            "xsh": xb[lo:hi],
            "ys": np.ascontiguousarray(y[lo:hi]).astype(ml_dtypes.bfloat16),
            "lnm": _make_mask(neg_indices[lo:hi]),
            "idf8": idf8,
        })
    return in_maps


def _ensure_ntff_hook():
    """Register the axon NTFF profile hook (missing from this image's antenv)."""
    import types, ctypes, contextlib
    try:
        from antenv.axon_hooks import get_axon_ntff_profile_hook  # noqa
        return
    except ImportError:
        pass
    so_path = "/opt/axon/libaxon_pjrt.so"
    import os
    if not os.path.exists(so_path):
        return
    lib = ctypes.CDLL(so_path)
    if not hasattr(lib, "axon_start_nrt_profile"):
        return
    lib.axon_start_nrt_profile.argtypes = [ctypes.POINTER(ctypes.c_int64),
                                           ctypes.c_size_t]
    lib.axon_start_nrt_profile.restype = ctypes.c_int64
    lib.axon_stop_nrt_profile.argtypes = [ctypes.c_char_p]
    lib.axon_stop_nrt_profile.restype = ctypes.c_int64

    @contextlib.contextmanager
    def _hook(output_dir, device_ids):
        import jax
        jax.devices()
        if device_ids:
            ids = (ctypes.c_int64 * len(device_ids))(*device_ids)
            rc = lib.axon_start_nrt_profile(ids, len(device_ids))
        else:
            rc = lib.axon_start_nrt_profile(None, 0)
        if rc != 0:
            raise RuntimeError(f"axon_start_nrt_profile rc={rc}")
        try:
            yield
        finally:
            n = lib.axon_stop_nrt_profile(str(output_dir).encode())
            if n < 0:
                raise RuntimeError(f"axon_stop_nrt_profile rc={n}")
            print(f"profile: {n} file(s) written to {output_dir}")

    mod = types.ModuleType("antenv.axon_hooks")
    _state = {"hook": _hook}
    mod.get_axon_ntff_profile_hook = lambda: _state["hook"]
    mod.set_axon_ntff_profile_hook = lambda h: _state.update(hook=h)
    import antenv
    sys.modules["antenv.axon_hooks"] = mod
    antenv.axon_hooks = mod


def run_spmd(in_maps, trace=False, **kw):
    nc = _get_program()
    if trace:
        _ensure_ntff_hook()
    return run_bass_kernel_spmd(nc, in_maps, list(range(N_CORES)), trace=trace, **kw)


def kernel(x, y, neg_indices):
    x = np.asarray(x)
    y = np.asarray(y)
    neg_indices = np.asarray(neg_indices)
    res = run_spmd(make_in_maps(x, y, neg_indices)).results
    losses = np.stack([res[c]["loss"] for c in range(N_CORES)])  # [8, 128, SLABS]
    return np.float32(losses.mean())


# revision 22
# speedup vs baseline: 1.2187x; 1.2187x over previous
"""Contrastive loss kernel for Trainium2, 8 NeuronCores, data-parallel over node rows.

v2 strategy (per core c, shard rows R_c = c*1024 .. c*1024+1024), gather-free:
  - Gram matrix in fp8(e4m3) with DoubleRow matmuls (K=256 folded as 128x2).
    The DR pair axis maps d = 2*q + j, which falls out of a uint16-viewed DMA
    transpose of fp8 element pairs -- no separate cast pass.
  - Mask application ON THE PE: an identity-weights fp8 matmul accumulates the
    ln-mask (0 selected / -20 unselected) straight into the Gram PSUM chunk, so
    selection is a single ACT pass: exp(2*(sim+mask)) with accum_out giving the
    per-row masked sums directly from PSUM.  A few chunks per column use the
    DVE add-first path instead to balance engine load.
  - Full-x normalize streamed in groups; rsqrt via DVE polynomial + one Newton
    step (SS/256 in [0.66,1.37]) so the Scalar activation table never thrashes
    between Sqrt and Exp mid-stream.
  - Chunk-column-major slab loop so Gram consumption tracks the group stream.
  - Positives: fused DVE dots on bf16 shard tiles; ln(pos) taken analytically.
  - loss = ln(pos + neg + eps) - 2*pos_arg; host averages the 8 cores' rows.
"""
import sys

sys.path.insert(0, "/opt/trn_rl_repo")

from contextlib import ExitStack

import numpy as np
import ml_dtypes

import concourse.bacc as bacc
import concourse.mybir as mybir
import concourse.tile as tile
from concourse.bass_utils import run_bass_kernel_spmd

N_NODES = 8192
D = 256
K_NEG = 64
N_CORES = 8
RPC = N_NODES // N_CORES      # rows per core = 1024
SLABS = RPC // 128            # 8 slabs of 128 rows
NT = N_NODES // 128           # 64 x-tiles of [128, 256]
NTS = RPC // 128              # 8 shard tiles
G = 8                         # tiles per x-group
NG = NT // G                  # 8 groups
CHUNK = 2048                  # Gram chunk (4 PSUM banks)
NCH = N_NODES // CHUNK        # 4 chunks per slab row
TAU_INV = 2.0                 # 1/(0.5 + 1e-10) ~= 2.0
EPS = 1e-5

F32 = mybir.dt.float32
BF16 = mybir.dt.bfloat16
FP8 = mybir.dt.float8e4
U16 = mybir.dt.uint16
MASK_NP = ml_dtypes.float8_e4m3

# per-chunk selection mode, keyed (slab, chunk).
# mask m01inv is 1.0 for UNSELECTED cols, 0.0 for negatives (and fp8-exact).
#   S: PE adds -20*m01inv via (-20*I)-matmul; ACT exp(2*in) from PSUM w/ accum.
#   V: DVE scalar_tensor_tensor Y = -20*m01inv + sim (bf16); ACT exp w/ accum.
# Early columns lean S (DVE busy with preamble); later columns lean V.
_MODE_COLS = [
    "SSSSSSSS",   # ch 0, slabs 0..7
    "SSSSSSSS",   # ch 1
    "SSSSSSSS",   # ch 2
    "SSSVSSVS",   # ch 3
]
MODE = {(s, ch): _MODE_COLS[ch][s] for ch in range(4) for s in range(8)}

_PROG = None


def _poly_rsqrt(nc, big, SSg, Rout):
    """Rout = 1/sqrt(SSg) for SSg ~ 256, DVE-only (no ACT table).

    a = SS/256; t = a-1; y0 = 1 - t/2 + 0.375 t^2; one Newton step.
    Scaled: 1/sqrt(SS) = (1/16) / sqrt(a) -> fold 1/16 into the last mul.
    """
    ALU = mybir.AluOpType
    shp = list(SSg.shape)
    t = big.tile(shp, F32, tag="rs_t")
    nc.vector.tensor_scalar(out=t, in0=SSg, scalar1=1.0 / 256.0, scalar2=-1.0,
                            op0=ALU.mult, op1=ALU.add)
    u = big.tile(shp, F32, tag="rs_u")
    nc.vector.tensor_scalar(out=u, in0=t, scalar1=0.375, scalar2=-0.5,
                            op0=ALU.mult, op1=ALU.add)
    w = big.tile(shp, F32, tag="rs_w")
    nc.vector.tensor_mul(w, u, t)
    y0 = big.tile(shp, F32, tag="rs_y0")
    nc.vector.tensor_scalar(out=y0, in0=w, scalar1=1.0, scalar2=None,
                            op0=ALU.add)
    b = big.tile(shp, F32, tag="rs_b")
    nc.vector.tensor_scalar(out=b, in0=SSg, scalar1=1.0 / 512.0, scalar2=None,
                            op0=ALU.mult)
    c = big.tile(shp, F32, tag="rs_c")
    nc.vector.tensor_mul(c, y0, y0)
    d = big.tile(shp, F32, tag="rs_d")
    nc.vector.tensor_mul(d, b, c)
    e = big.tile(shp, F32, tag="rs_e")
    nc.vector.tensor_scalar(out=e, in0=d, scalar1=-1.0, scalar2=1.5,
                            op0=ALU.mult, op1=ALU.add)
    y1 = big.tile(shp, F32, tag="rs_y1")
    nc.vector.tensor_mul(y1, y0, e)
    # R = y1 / 16
    nc.vector.tensor_scalar(out=Rout, in0=y1, scalar1=1.0 / 16.0, scalar2=None,
                            op0=ALU.mult)


def _build_program():
    nc = bacc.Bacc("TRN2", target_bir_lowering=False, debug=False,
                   num_devices=N_CORES)

    xb_d = nc.dram_tensor("xb", [N_NODES, D], BF16, kind="ExternalInput")
    xsh_d = nc.dram_tensor("xsh", [RPC, D], BF16, kind="ExternalInput")
    ys_d = nc.dram_tensor("ys", [RPC, D], BF16, kind="ExternalInput")
    lnm_d = nc.dram_tensor("lnm", [128, SLABS, N_NODES], FP8,
                           kind="ExternalInput")
    id_d = nc.dram_tensor("idf8", [128, 128], FP8, kind="ExternalInput")
    loss_d = nc.dram_tensor("loss", [128, SLABS], F32, kind="ExternalOutput")

    AF = mybir.ActivationFunctionType
    ALU = mybir.AluOpType
    DR = mybir.MatmulPerfMode.DoubleRow

    with tile.TileContext(nc) as tc, ExitStack() as ctx:
        big = ctx.enter_context(tc.tile_pool(name="big", bufs=1))
        sqpool = ctx.enter_context(tc.tile_pool(name="sqpool", bufs=1))
        zpool = ctx.enter_context(tc.tile_pool(name="zpool", bufs=2))
        ypool = ctx.enter_context(tc.tile_pool(name="ypool", bufs=2))
        expool = ctx.enter_context(tc.tile_pool(name="expool", bufs=2))
        psum = ctx.enter_context(tc.tile_pool(name="psum", bufs=2, space="PSUM"))

        # ---------------- input DMAs (x before masks!) ----------------
        xshb = big.tile([128, NTS, D], BF16)
        nc.sync.dma_start(out=xshb,
                          in_=xsh_d.ap().rearrange("(t p) d -> p t d", p=128))
        ysb = big.tile([128, NTS, D], BF16)
        nc.sync.dma_start(out=ysb,
                          in_=ys_d.ap().rearrange("(t p) d -> p t d", p=128))
        idsb = big.tile([128, 128], FP8)
        nc.sync.dma_start(out=idsb, in_=id_d.ap())
        x_r = xb_d.ap().rearrange("(t p) d -> p t d", p=128)
        xall = big.tile([128, NT, D], BF16)
        lnm = big.tile([128, SLABS, N_NODES], FP8)
        # masks go on the scalar HWDGE queue (ACT idle early) so the sync
        # queue's transposes are not head-of-line blocked behind 8MB of
        # mask traffic.  xall g2..g7 dispatch inside the group loop.
        for s in range(SLABS):
            nc.scalar.dma_start(out=lnm[:, s, :], in_=lnm_d.ap()[:, s, :])
        for g in range(2):
            nc.sync.dma_start(out=xall[:, g * G:(g + 1) * G, :],
                              in_=x_r[:, g * G:(g + 1) * G, :])

        # ---------------- shard norms + positives ----------------
        SSx = big.tile([128, NTS], F32)
        SSy = big.tile([128, NTS], F32)
        sqs = sqpool.tile([128, NTS, D], BF16, tag="sqs")
        nc.vector.tensor_mul(sqs.rearrange("p a b -> p (a b)"),
                             xshb.rearrange("p a b -> p (a b)"),
                             xshb.rearrange("p a b -> p (a b)"))
        nc.vector.tensor_reduce(out=SSx, in_=sqs, axis=mybir.AxisListType.X,
                                op=ALU.add)
        sqy = sqpool.tile([128, NTS, D], BF16, tag="sqs")
        nc.vector.tensor_mul(sqy.rearrange("p a b -> p (a b)"),
                             ysb.rearrange("p a b -> p (a b)"),
                             ysb.rearrange("p a b -> p (a b)"))
        nc.vector.tensor_reduce(out=SSy, in_=sqy, axis=mybir.AxisListType.X,
                                op=ALU.add)
        Rx = big.tile([128, NTS], F32)
        _poly_rsqrt(nc, zpool, SSx, Rx)
        Ry = big.tile([128, NTS], F32)
        _poly_rsqrt(nc, zpool, SSy, Ry)

        # positive dots
        DXY = big.tile([128, NTS], F32)
        xy = sqpool.tile([128, NTS, D], BF16, tag="sqs")
        nc.vector.tensor_mul(xy.rearrange("p a b -> p (a b)"),
                             xshb.rearrange("p a b -> p (a b)"),
                             ysb.rearrange("p a b -> p (a b)"))
        nc.vector.tensor_reduce(out=DXY, in_=xy, axis=mybir.AxisListType.X,
                                op=ALU.add)
        PA = big.tile([128, NTS], F32)
        nc.vector.tensor_mul(PA, DXY, Rx)
        PA2 = big.tile([128, NTS], F32)
        nc.vector.tensor_mul(PA2, PA, Ry)
        POS = big.tile([128, NTS], F32)
        nc.scalar.activation(POS, PA2, AF.Exp, scale=TAU_INV)

        # ---------------- shard z (fp8 pairs) + transpose ----------------
        zsf8 = big.tile([128, NTS, D], FP8)
        for t in range(NTS):
            nc.vector.tensor_scalar(out=zsf8[:, t, :], in0=xshb[:, t, :],
                                    scalar1=Rx[:, t:t + 1], scalar2=None,
                                    op0=ALU.mult)
        z1sT = big.tile([128, 2 * RPC], FP8)   # u16 cols = RPC
        nc.sync.dma_start(
            out=z1sT.bitcast(U16).rearrange("p (b q) -> p b q", q=128),
            in_=zsf8.bitcast(U16).rearrange("p a b -> p (a b)"),
            transpose=True)
        # de-interleave pairs for the weights: LDWEIGHTS DoubleRow requires the
        # pair stride to be a multiple of 16 elements (s3_lw_dual_fp8), so the
        # adjacent-pair transpose layout is illegal for lhsT. One DVE copy.
        z1sT_w = big.tile([128, 2, RPC], FP8)
        nc.vector.tensor_copy(out=z1sT_w,
                              in_=z1sT.rearrange("p (n j) -> p j n", j=2))

        # ---------------- full-x: norms (DVE rsqrt) + scale + transpose ----
        z1T = big.tile([128, 2 * N_NODES], FP8)  # u16 cols = N_NODES
        R = big.tile([128, NT], F32)
        SS = big.tile([128, NT], F32)
        for g in range(NG):
            sl = slice(g * G, (g + 1) * G)
            if g + 2 < NG:
                g2 = g + 2
                nc.sync.dma_start(out=xall[:, g2 * G:(g2 + 1) * G, :],
                                  in_=x_r[:, g2 * G:(g2 + 1) * G, :])
            sq = sqpool.tile([128, G, D], BF16, tag="sqg")
            nc.vector.tensor_mul(sq.rearrange("p a b -> p (a b)"),
                                 xall[:, sl, :].rearrange("p a b -> p (a b)"),
                                 xall[:, sl, :].rearrange("p a b -> p (a b)"))
            nc.vector.tensor_reduce(out=SS[:, sl], in_=sq,
                                    axis=mybir.AxisListType.X, op=ALU.add)
            _poly_rsqrt(nc, zpool, SS[:, sl], R[:, sl])
            zf8 = zpool.tile([128, G, D], FP8, tag="zf8")
            for tl in range(G):
                t = g * G + tl
                nc.vector.tensor_scalar(out=zf8[:, tl, :], in0=xall[:, t, :],
                                        scalar1=R[:, t:t + 1], scalar2=None,
                                        op0=ALU.mult)
            nc.sync.dma_start(
                out=z1T.bitcast(U16)[:, g * G * 128:(g + 1) * G * 128]
                       .rearrange("p (b q) -> p b q", q=128),
                in_=zf8.bitcast(U16).rearrange("p a b -> p (a b)"),
                transpose=True)
        z1T_dr = z1T.rearrange("p (n j) -> p j n", j=2)  # [128, 2, N_NODES]

        # ---------------- selection: chunk-column-major slab loop ----------
        NEGC = big.tile([128, SLABS * NCH], F32)
        nc.vector.memset(NEGC, 0.0)
        for ch in range(NCH):
            for s in range(SLABS):
                mode = MODE[(s, ch)]
                lhsT = z1sT_w[:, :, s * 128:(s + 1) * 128]
                acc = NEGC[:, s * NCH + ch:s * NCH + ch + 1]
                msk = lnm[:, s, ch * CHUNK:(ch + 1) * CHUNK]
                ps = psum.tile([128, CHUNK], F32, tag="ps")
                for j in range(CHUNK // 512):
                    col = ch * CHUNK + j * 512
                    nc.tensor.matmul(ps[:, j * 512:(j + 1) * 512],
                                     lhsT=lhsT,
                                     rhs=z1T_dr[:, :, col:col + 512],
                                     start=True, stop=(mode != "S"),
                                     perf_mode=DR)
                if mode == "S":
                    # PSUM += 20*m01 via identity matmul; exp(2*in - 40)
                    for j in range(CHUNK // 512):
                        col = ch * CHUNK + j * 512
                        nc.tensor.matmul(ps[:, j * 512:(j + 1) * 512],
                                         lhsT=idsb,
                                         rhs=lnm[:, s, col:col + 512],
                                         start=False, stop=True)
                    ex = expool.tile([128, CHUNK], BF16, tag="ex")
                    nc.scalar.activation(ex, ps, AF.Exp, scale=TAU_INV,
                                         accum_out=acc)
                elif mode == "V":
                    Y = ypool.tile([128, CHUNK], BF16, tag="y")
                    nc.vector.scalar_tensor_tensor(
                        out=Y, in0=msk, scalar=-20.0, in1=ps,
                        op0=ALU.mult, op1=ALU.add)
                    ex = expool.tile([128, CHUNK], BF16, tag="ex")
                    nc.scalar.activation(ex, Y, AF.Exp, scale=TAU_INV,
                                         accum_out=acc)

        # ---------------- loss assembly ----------------
        NEG = big.tile([128, SLABS], F32)
        nc.vector.tensor_reduce(
            out=NEG, in_=NEGC.rearrange("p (s c) -> p s c", c=NCH),
            axis=mybir.AxisListType.X, op=ALU.add)
        DEN = big.tile([128, SLABS], F32)
        nc.vector.tensor_add(DEN, NEG, POS)
        DEN2 = big.tile([128, SLABS], F32)
        nc.vector.tensor_scalar_add(DEN2, DEN, EPS)
        LD = big.tile([128, SLABS], F32)
        nc.scalar.activation(LD, DEN2, AF.Ln)
        LP = big.tile([128, SLABS], F32)
        nc.vector.tensor_scalar_mul(LP, PA2, TAU_INV)
        LOSS = big.tile([128, SLABS], F32)
        nc.vector.tensor_sub(LOSS, LD, LP)
        nc.sync.dma_start(out=loss_d.ap(), in_=LOSS)

    nc.compile()
    return nc


def _get_program():
    global _PROG
    if _PROG is None:
        _PROG = _build_program()
    return _PROG


def _make_mask(idx_core: np.ndarray) -> np.ndarray:
    """[1024, 64] int -> [128, 8, 8192] fp8 0/1 mask (p, slab, col)."""
    idxc = idx_core.reshape(SLABS, 128, K_NEG).transpose(1, 0, 2)  # [p, s, k]
    m01 = np.ones((128, SLABS, N_NODES), dtype=np.float32)
    pp = np.arange(128)[:, None, None]
    ss = np.arange(SLABS)[None, :, None]
    m01[pp, ss, idxc] = 0.0
    return m01.astype(MASK_NP)


def make_in_maps(x, y, neg_indices):
    xb = np.ascontiguousarray(x).astype(ml_dtypes.bfloat16)
    idf8 = (-20.0 * np.eye(128, dtype=np.float32)).astype(MASK_NP)
    in_maps = []
    for c in range(N_CORES):
        lo, hi = c * RPC, (c + 1) * RPC
        in_maps.append({
            "xb": xb,
            "xsh": xb[lo:hi],
            "ys": np.ascontiguousarray(y[lo:hi]).astype(ml_dtypes.bfloat16),
            "lnm": _make_mask(neg_indices[lo:hi]),
            "idf8": idf8,
        })
    return in_maps


def _ensure_ntff_hook():
    """Register the axon NTFF profile hook (missing from this image's antenv)."""
    import types, ctypes, contextlib
    try:
        from antenv.axon_hooks import get_axon_ntff_profile_hook  # noqa
        return
    except ImportError:
        pass
    so_path = "/opt/axon/libaxon_pjrt.so"
    import os
    if not os.path.exists(so_path):
        return
    lib = ctypes.CDLL(so_path)
    if not hasattr(lib, "axon_start_nrt_profile"):
        return
    lib.axon_start_nrt_profile.argtypes = [ctypes.POINTER(ctypes.c_int64),
                                           ctypes.c_size_t]
    lib.axon_start_nrt_profile.restype = ctypes.c_int64
    lib.axon_stop_nrt_profile.argtypes = [ctypes.c_char_p]
    lib.axon_stop_nrt_profile.restype = ctypes.c_int64

    @contextlib.contextmanager
    def _hook(output_dir, device_ids):
        import jax
        jax.devices()
        if device_ids:
            ids = (ctypes.c_int64 * len(device_ids))(*device_ids)
            rc = lib.axon_start_nrt_profile(ids, len(device_ids))
        else:
            rc = lib.axon_start_nrt_profile(None, 0)
        if rc != 0:
            raise RuntimeError(f"axon_start_nrt_profile rc={rc}")
        try:
            yield
        finally:
            n = lib.axon_stop_nrt_profile(str(output_dir).encode())
            if n < 0:
                raise RuntimeError(f"axon_stop_nrt_profile rc={n}")
            print(f"profile: {n} file(s) written to {output_dir}")

    mod = types.ModuleType("antenv.axon_hooks")
    _state = {"hook": _hook}
    mod.get_axon_ntff_profile_hook = lambda: _state["hook"]
    mod.set_axon_ntff_profile_hook = lambda h: _state.update(hook=h)
    import antenv
    sys.modules["antenv.axon_hooks"] = mod
    antenv.axon_hooks = mod


def run_spmd(in_maps, trace=False, **kw):
    nc = _get_program()
    if trace:
        _ensure_ntff_hook()
    return run_bass_kernel_spmd(nc, in_maps, list(range(N_CORES)), trace=trace, **kw)


def kernel(x, y, neg_indices):
    x = np.asarray(x)
    y = np.asarray(y)
    neg_indices = np.asarray(neg_indices)
    res = run_spmd(make_in_maps(x, y, neg_indices)).results
    losses = np.stack([res[c]["loss"] for c in range(N_CORES)])  # [8, 128, SLABS]
    return np.float32(losses.mean())


# revision 23
# speedup vs baseline: 1.2966x; 1.0639x over previous
"""Contrastive loss kernel for Trainium2, 8 NeuronCores, data-parallel over node rows.

v2 strategy (per core c, shard rows R_c = c*1024 .. c*1024+1024), gather-free:
  - Gram matrix in fp8(e4m3) with DoubleRow matmuls (K=256 folded as 128x2).
    The DR pair axis maps d = 2*q + j, which falls out of a uint16-viewed DMA
    transpose of fp8 element pairs -- no separate cast pass.
  - Mask application ON THE PE: an identity-weights fp8 matmul accumulates the
    ln-mask (0 selected / -20 unselected) straight into the Gram PSUM chunk, so
    selection is a single ACT pass: exp(2*(sim+mask)) with accum_out giving the
    per-row masked sums directly from PSUM.  A few chunks per column use the
    DVE add-first path instead to balance engine load.
  - Full-x normalize streamed in groups; rsqrt via DVE polynomial + one Newton
    step (SS/256 in [0.66,1.37]) so the Scalar activation table never thrashes
    between Sqrt and Exp mid-stream.
  - Chunk-column-major slab loop so Gram consumption tracks the group stream.
  - Positives: fused DVE dots on bf16 shard tiles; ln(pos) taken analytically.
  - loss = ln(pos + neg + eps) - 2*pos_arg; host averages the 8 cores' rows.
"""
import sys

sys.path.insert(0, "/opt/trn_rl_repo")

from contextlib import ExitStack

import numpy as np
import ml_dtypes

import concourse.bacc as bacc
import concourse.mybir as mybir
import concourse.tile as tile
from concourse.bass_utils import run_bass_kernel_spmd

N_NODES = 8192
D = 256
K_NEG = 64
N_CORES = 8
RPC = N_NODES // N_CORES      # rows per core = 1024
SLABS = RPC // 128            # 8 slabs of 128 rows
NT = N_NODES // 128           # 64 x-tiles of [128, 256]
NTS = RPC // 128              # 8 shard tiles
G = 8                         # tiles per x-group
NG = NT // G                  # 8 groups
CHUNK = 2048                  # Gram chunk (4 PSUM banks)
NCH = N_NODES // CHUNK        # 4 chunks per slab row
TAU_INV = 2.0                 # 1/(0.5 + 1e-10) ~= 2.0
EPS = 1e-5

F32 = mybir.dt.float32
BF16 = mybir.dt.bfloat16
FP8 = mybir.dt.float8e4
U16 = mybir.dt.uint16
MASK_NP = ml_dtypes.float8_e4m3

# per-chunk selection mode, keyed (slab, chunk).
# mask m01inv is 1.0 for UNSELECTED cols, 0.0 for negatives (and fp8-exact).
#   S: PE adds -20*m01inv via (-20*I)-matmul; ACT exp(2*in) from PSUM w/ accum.
#   V: DVE scalar_tensor_tensor Y = -20*m01inv + sim (bf16); ACT exp w/ accum.
# Early columns lean S (DVE busy with preamble); later columns lean V.
_MODE_COLS = [
    "SSSSSSSS",   # ch 0, slabs 0..7
    "SSSSSSSS",   # ch 1
    "SSSSSSSS",   # ch 2
    "SSSVSSVS",   # ch 3
]
MODE = {(s, ch): _MODE_COLS[ch][s] for ch in range(4) for s in range(8)}

_PROG = None


def _poly_rsqrt(nc, big, SSg, Rout):
    """Rout = 1/sqrt(SSg) for SSg ~ 256, DVE-only (no ACT table).

    a = SS/256; t = a-1; y0 = 1 - t/2 + 0.375 t^2; one Newton step.
    Scaled: 1/sqrt(SS) = (1/16) / sqrt(a) -> fold 1/16 into the last mul.
    """
    ALU = mybir.AluOpType
    shp = list(SSg.shape)
    t = big.tile(shp, F32, tag="rs_t")
    nc.vector.tensor_scalar(out=t, in0=SSg, scalar1=1.0 / 256.0, scalar2=-1.0,
                            op0=ALU.mult, op1=ALU.add)
    u = big.tile(shp, F32, tag="rs_u")
    nc.vector.tensor_scalar(out=u, in0=t, scalar1=0.375, scalar2=-0.5,
                            op0=ALU.mult, op1=ALU.add)
    w = big.tile(shp, F32, tag="rs_w")
    nc.vector.tensor_mul(w, u, t)
    y0 = big.tile(shp, F32, tag="rs_y0")
    nc.vector.tensor_scalar(out=y0, in0=w, scalar1=1.0, scalar2=None,
                            op0=ALU.add)
    b = big.tile(shp, F32, tag="rs_b")
    nc.vector.tensor_scalar(out=b, in0=SSg, scalar1=1.0 / 512.0, scalar2=None,
                            op0=ALU.mult)
    c = big.tile(shp, F32, tag="rs_c")
    nc.vector.tensor_mul(c, y0, y0)
    d = big.tile(shp, F32, tag="rs_d")
    nc.vector.tensor_mul(d, b, c)
    e = big.tile(shp, F32, tag="rs_e")
    nc.vector.tensor_scalar(out=e, in0=d, scalar1=-1.0, scalar2=1.5,
                            op0=ALU.mult, op1=ALU.add)
    y1 = big.tile(shp, F32, tag="rs_y1")
    nc.vector.tensor_mul(y1, y0, e)
    # R = y1 / 16
    nc.vector.tensor_scalar(out=Rout, in0=y1, scalar1=1.0 / 16.0, scalar2=None,
                            op0=ALU.mult)


def _build_program():
    nc = bacc.Bacc("TRN2", target_bir_lowering=False, debug=False,
                   num_devices=N_CORES)

    xb_d = nc.dram_tensor("xb", [N_NODES, D], BF16, kind="ExternalInput")
    xsh_d = nc.dram_tensor("xsh", [RPC, D], BF16, kind="ExternalInput")
    ys_d = nc.dram_tensor("ys", [RPC, D], BF16, kind="ExternalInput")
    lnm_d = nc.dram_tensor("lnm", [128, SLABS, N_NODES], FP8,
                           kind="ExternalInput")
    id_d = nc.dram_tensor("idf8", [128, 128], FP8, kind="ExternalInput")
    loss_d = nc.dram_tensor("loss", [128, SLABS], F32, kind="ExternalOutput")

    AF = mybir.ActivationFunctionType
    ALU = mybir.AluOpType
    DR = mybir.MatmulPerfMode.DoubleRow

    with tile.TileContext(nc) as tc, ExitStack() as ctx:
        big = ctx.enter_context(tc.tile_pool(name="big", bufs=1))
        sqpool = ctx.enter_context(tc.tile_pool(name="sqpool", bufs=1))
        zpool = ctx.enter_context(tc.tile_pool(name="zpool", bufs=2))
        ypool = ctx.enter_context(tc.tile_pool(name="ypool", bufs=2))
        expool = ctx.enter_context(tc.tile_pool(name="expool", bufs=2))
        psum = ctx.enter_context(tc.tile_pool(name="psum", bufs=2, space="PSUM"))

        # ---------------- input DMAs (x before masks!) ----------------
        xshb = big.tile([128, NTS, D], BF16)
        nc.sync.dma_start(out=xshb,
                          in_=xsh_d.ap().rearrange("(p t) d -> p t d", t=NTS))
        ysb = big.tile([128, NTS, D], BF16)
        nc.sync.dma_start(out=ysb,
                          in_=ys_d.ap().rearrange("(p t) d -> p t d", t=NTS))
        idsb = big.tile([128, 128], FP8)
        nc.sync.dma_start(out=idsb, in_=id_d.ap())
        x_r = xb_d.ap().rearrange("(p t) d -> p t d", t=NT)
        xall = big.tile([128, NT, D], BF16)
        lnm = big.tile([128, SLABS, N_NODES], FP8)
        # masks go on the scalar HWDGE queue (ACT idle early) so the sync
        # queue's transposes are not head-of-line blocked behind 8MB of
        # mask traffic.  xall quarters q2,q3 dispatch inside the group loop.
        for s in range(SLABS):
            nc.scalar.dma_start(out=lnm[:, s, :], in_=lnm_d.ap()[:, s, :])
        for q in range(2):
            nc.sync.dma_start(out=xall[:, q * 16:(q + 1) * 16, :],
                              in_=x_r[:, q * 16:(q + 1) * 16, :])

        # ---------------- shard norms + positives ----------------
        SSx = big.tile([128, NTS], F32)
        SSy = big.tile([128, NTS], F32)
        sqs = sqpool.tile([128, NTS, D], BF16, tag="sqs")
        nc.vector.tensor_mul(sqs.rearrange("p a b -> p (a b)"),
                             xshb.rearrange("p a b -> p (a b)"),
                             xshb.rearrange("p a b -> p (a b)"))
        nc.vector.tensor_reduce(out=SSx, in_=sqs, axis=mybir.AxisListType.X,
                                op=ALU.add)
        sqy = sqpool.tile([128, NTS, D], BF16, tag="sqs")
        nc.vector.tensor_mul(sqy.rearrange("p a b -> p (a b)"),
                             ysb.rearrange("p a b -> p (a b)"),
                             ysb.rearrange("p a b -> p (a b)"))
        nc.vector.tensor_reduce(out=SSy, in_=sqy, axis=mybir.AxisListType.X,
                                op=ALU.add)
        Rx = big.tile([128, NTS], F32)
        _poly_rsqrt(nc, zpool, SSx, Rx)
        Ry = big.tile([128, NTS], F32)
        _poly_rsqrt(nc, zpool, SSy, Ry)

        # positive dots
        DXY = big.tile([128, NTS], F32)
        xy = sqpool.tile([128, NTS, D], BF16, tag="sqs")
        nc.vector.tensor_mul(xy.rearrange("p a b -> p (a b)"),
                             xshb.rearrange("p a b -> p (a b)"),
                             ysb.rearrange("p a b -> p (a b)"))
        nc.vector.tensor_reduce(out=DXY, in_=xy, axis=mybir.AxisListType.X,
                                op=ALU.add)
        PA = big.tile([128, NTS], F32)
        nc.vector.tensor_mul(PA, DXY, Rx)
        PA2 = big.tile([128, NTS], F32)
        nc.vector.tensor_mul(PA2, PA, Ry)
        POS = big.tile([128, NTS], F32)
        nc.scalar.activation(POS, PA2, AF.Exp, scale=TAU_INV)

        # ---------------- shard z (fp8 pairs) + transpose ----------------
        zsf8 = big.tile([128, NTS, D], FP8)
        for t in range(NTS):
            nc.vector.tensor_scalar(out=zsf8[:, t, :], in0=xshb[:, t, :],
                                    scalar1=Rx[:, t:t + 1], scalar2=None,
                                    op0=ALU.mult)
        z1sT = big.tile([128, 2 * RPC], FP8)   # u16 cols = RPC
        nc.sync.dma_start(
            out=z1sT.bitcast(U16).rearrange("p (b q) -> p b q", q=128),
            in_=zsf8.bitcast(U16).rearrange("p a b -> p (a b)"),
            transpose=True)
        # de-interleave pairs for the weights: LDWEIGHTS DoubleRow requires the
        # pair stride to be a multiple of 16 elements (s3_lw_dual_fp8), so the
        # adjacent-pair transpose layout is illegal for lhsT. One DVE copy.
        z1sT_w = big.tile([128, 2, RPC], FP8)
        nc.vector.tensor_copy(out=z1sT_w,
                              in_=z1sT.rearrange("p (n j) -> p j n", j=2))

        # ---------------- full-x: norms (DVE rsqrt) + scale + transpose ----
        z1T = big.tile([128, 2 * N_NODES], FP8)  # u16 cols = N_NODES
        R = big.tile([128, NT], F32)
        SS = big.tile([128, NT], F32)
        for g in range(NG):
            sl = slice(g * G, (g + 1) * G)
            if g in (0, 2):
                q = g // 2 + 2
                nc.sync.dma_start(out=xall[:, q * 16:(q + 1) * 16, :],
                                  in_=x_r[:, q * 16:(q + 1) * 16, :])
            sq = sqpool.tile([128, G, D], BF16, tag="sqg")
            nc.vector.tensor_mul(sq.rearrange("p a b -> p (a b)"),
                                 xall[:, sl, :].rearrange("p a b -> p (a b)"),
                                 xall[:, sl, :].rearrange("p a b -> p (a b)"))
            nc.vector.tensor_reduce(out=SS[:, sl], in_=sq,
                                    axis=mybir.AxisListType.X, op=ALU.add)
            _poly_rsqrt(nc, zpool, SS[:, sl], R[:, sl])
            zf8 = zpool.tile([128, G, D], FP8, tag="zf8")
            for tl in range(G):
                t = g * G + tl
                nc.vector.tensor_scalar(out=zf8[:, tl, :], in0=xall[:, t, :],
                                        scalar1=R[:, t:t + 1], scalar2=None,
                                        op0=ALU.mult)
            nc.sync.dma_start(
                out=z1T.bitcast(U16)[:, g * G * 128:(g + 1) * G * 128]
                       .rearrange("p (b q) -> p b q", q=128),
                in_=zf8.bitcast(U16).rearrange("p a b -> p (a b)"),
                transpose=True)
        z1T_dr = z1T.rearrange("p (n j) -> p j n", j=2)  # [128, 2, N_NODES]

        # ---------------- selection: chunk-column-major slab loop ----------
        NEGC = big.tile([128, SLABS * NCH], F32)
        nc.vector.memset(NEGC, 0.0)
        for ch in range(NCH):
            for s in range(SLABS):
                mode = MODE[(s, ch)]
                lhsT = z1sT_w[:, :, s * 128:(s + 1) * 128]
                acc = NEGC[:, s * NCH + ch:s * NCH + ch + 1]
                msk = lnm[:, s, ch * CHUNK:(ch + 1) * CHUNK]
                ps = psum.tile([128, CHUNK], F32, tag="ps")
                for j in range(CHUNK // 512):
                    col = ch * CHUNK + j * 512
                    nc.tensor.matmul(ps[:, j * 512:(j + 1) * 512],
                                     lhsT=lhsT,
                                     rhs=z1T_dr[:, :, col:col + 512],
                                     start=True, stop=(mode != "S"),
                                     perf_mode=DR)
                if mode == "S":
                    # PSUM += 20*m01 via identity matmul; exp(2*in - 40)
                    for j in range(CHUNK // 512):
                        col = ch * CHUNK + j * 512
                        nc.tensor.matmul(ps[:, j * 512:(j + 1) * 512],
                                         lhsT=idsb,
                                         rhs=lnm[:, s, col:col + 512],
                                         start=False, stop=True)
                    ex = expool.tile([128, CHUNK], BF16, tag="ex")
                    nc.scalar.activation(ex, ps, AF.Exp, scale=TAU_INV,
                                         accum_out=acc)
                elif mode == "V":
                    Y = ypool.tile([128, CHUNK], BF16, tag="y")
                    nc.vector.scalar_tensor_tensor(
                        out=Y, in0=msk, scalar=-20.0, in1=ps,
                        op0=ALU.mult, op1=ALU.add)
                    ex = expool.tile([128, CHUNK], BF16, tag="ex")
                    nc.scalar.activation(ex, Y, AF.Exp, scale=TAU_INV,
                                         accum_out=acc)

        # ---------------- loss assembly ----------------
        NEG = big.tile([128, SLABS], F32)
        nc.vector.tensor_reduce(
            out=NEG, in_=NEGC.rearrange("p (s c) -> p s c", c=NCH),
            axis=mybir.AxisListType.X, op=ALU.add)
        DEN = big.tile([128, SLABS], F32)
        nc.vector.tensor_add(DEN, NEG, POS)
        DEN2 = big.tile([128, SLABS], F32)
        nc.vector.tensor_scalar_add(DEN2, DEN, EPS)
        LD = big.tile([128, SLABS], F32)
        nc.scalar.activation(LD, DEN2, AF.Ln)
        LP = big.tile([128, SLABS], F32)
        nc.vector.tensor_scalar_mul(LP, PA2, TAU_INV)
        LOSS = big.tile([128, SLABS], F32)
        nc.vector.tensor_sub(LOSS, LD, LP)
        nc.sync.dma_start(out=loss_d.ap(), in_=LOSS)

    nc.compile()
    return nc


def _get_program():
    global _PROG
    if _PROG is None:
        _PROG = _build_program()
    return _PROG


# partition-major node mapping: SBUF tile (p, t) holds node p*NT_PER + t, so
# Gram column c = t*128 + p corresponds to node (c % 128)*NT_PER + c // 128.
# Slab s row at psum partition p is shard node p*SLABS + s.
_COL2NODE = (np.arange(N_NODES) % 128) * (N_NODES // 128) + \
    np.arange(N_NODES) // 128


def _make_mask(idx_core: np.ndarray) -> np.ndarray:
    """[1024, 64] int -> [128, 8, 8192] fp8 0/1 inv-mask, permuted layout."""
    # row for (p, s) = p*SLABS + s
    rows = (np.arange(128)[:, None] * SLABS +
            np.arange(SLABS)[None, :])             # [p, s]
    idxc = idx_core[rows]                          # [p, s, k] node ids
    m01 = np.ones((128, SLABS, N_NODES), dtype=np.float32)
    pp = np.arange(128)[:, None, None]
    ss = np.arange(SLABS)[None, :, None]
    m01[pp, ss, idxc] = 0.0                        # node-indexed columns
    return np.ascontiguousarray(m01[:, :, _COL2NODE]).astype(MASK_NP)


def make_in_maps(x, y, neg_indices):
    xb = np.ascontiguousarray(x).astype(ml_dtypes.bfloat16)
    idf8 = (-20.0 * np.eye(128, dtype=np.float32)).astype(MASK_NP)
    in_maps = []
    for c in range(N_CORES):
        lo, hi = c * RPC, (c + 1) * RPC
        in_maps.append({
            "xb": xb,
            "xsh": xb[lo:hi],
            "ys": np.ascontiguousarray(y[lo:hi]).astype(ml_dtypes.bfloat16),
            "lnm": _make_mask(neg_indices[lo:hi]),
            "idf8": idf8,
        })
    return in_maps


def _ensure_ntff_hook():
    """Register the axon NTFF profile hook (missing from this image's antenv)."""
    import types, ctypes, contextlib
    try:
        from antenv.axon_hooks import get_axon_ntff_profile_hook  # noqa
        return
    except ImportError:
        pass
    so_path = "/opt/axon/libaxon_pjrt.so"
    import os
    if not os.path.exists(so_path):
        return
    lib = ctypes.CDLL(so_path)
    if not hasattr(lib, "axon_start_nrt_profile"):
        return
    lib.axon_start_nrt_profile.argtypes = [ctypes.POINTER(ctypes.c_int64),
                                           ctypes.c_size_t]
    lib.axon_start_nrt_profile.restype = ctypes.c_int64
    lib.axon_stop_nrt_profile.argtypes = [ctypes.c_char_p]
    lib.axon_stop_nrt_profile.restype = ctypes.c_int64

    @contextlib.contextmanager
    def _hook(output_dir, device_ids):
        import jax
        jax.devices()
        if device_ids:
            ids = (ctypes.c_int64 * len(device_ids))(*device_ids)
            rc = lib.axon_start_nrt_profile(ids, len(device_ids))
        else:
            rc = lib.axon_start_nrt_profile(None, 0)
        if rc != 0:
            raise RuntimeError(f"axon_start_nrt_profile rc={rc}")
        try:
            yield
        finally:
            n = lib.axon_stop_nrt_profile(str(output_dir).encode())
            if n < 0:
                raise RuntimeError(f"axon_stop_nrt_profile rc={n}")
            print(f"profile: {n} file(s) written to {output_dir}")

    mod = types.ModuleType("antenv.axon_hooks")
    _state = {"hook": _hook}
    mod.get_axon_ntff_profile_hook = lambda: _state["hook"]
    mod.set_axon_ntff_profile_hook = lambda h: _state.update(hook=h)
    import antenv
    sys.modules["antenv.axon_hooks"] = mod
    antenv.axon_hooks = mod


def run_spmd(in_maps, trace=False, **kw):
    nc = _get_program()
    if trace:
        _ensure_ntff_hook()
    return run_bass_kernel_spmd(nc, in_maps, list(range(N_CORES)), trace=trace, **kw)


def kernel(x, y, neg_indices):
    x = np.asarray(x)
    y = np.asarray(y)
    neg_indices = np.asarray(neg_indices)
    res = run_spmd(make_in_maps(x, y, neg_indices)).results
    losses = np.stack([res[c]["loss"] for c in range(N_CORES)])  # [8, 128, SLABS]
    return np.float32(losses.mean())
